# revision 5
# baseline (speedup 1.0000x reference)
"""Trainium2 Bass kernel for AtomicMNISTClassifier (3-layer MLP + log_softmax).

Data-parallel across 8 NeuronCores: batch 32768 -> 4096 rows/core, weights
replicated. Host pre-processing casts x and the (transposed) weights to bf16
and zero-pads the 784 features to 896 = 7*128 so every contraction chunk is a
full 128 partitions.

Per-core pipeline, per 512-row batch chunk:
  7x dma_start_transpose: x[512,128] bf16 -> SBUF [128,512] feature-major
     (the DMA xbar does the transpose; no PE/DVE transpose needed)
  L1: 7 accumulating bf16 matmuls -> PSUM[128,512] fp32
      ScalarE epilogue relu(psum + b1) -> SBUF fp32
      GpSimd same-dtype strided copy of the bf16 high-half view -> bf16 tile
      (engine ops that *convert* f32->bf16 hang on this stack, so the
      truncation is done by byte-view instead)
  L2: 2 matmuls + relu epilogues -> h2 bf16 (same trick)
  L3: 2 accumulating matmuls -> PSUM[10,512]; bias epilogue -> fp32 logits
  log_softmax: TensorE transposes logits [10,128]->[128,10] per subtile;
      per-subtile max-reduce/subtract on VectorE; one batched Exp and one
      batched Ln on ScalarE per chunk; final subtract on VectorE
  DMA out[512,10] fp32 -> DRAM

All ScalarE functions used (Relu/Exp/Ln/Identity) share one ACT table set
(`natural_log_exp_and_others`), so no table reloads occur.
"""

import sys

for _p in ("/opt/trn_rl_repo",):
    if _p not in sys.path:
        sys.path.insert(0, _p)

import ml_dtypes
import numpy as np

import concourse.bass as bass
import concourse.tile as tile
from concourse import bacc, mybir
from concourse.bass import ts
from concourse.bass_utils import run_bass_kernel_spmd
from concourse.masks import make_identity

N_CORES = 8
B_FULL = 32768
B_SH = B_FULL // N_CORES  # 4096
F_IN = 784
F_PAD = 896  # 7 * 128
H1 = 128
H2 = 256
NCLS = 10
CHUNK_B = 512
N_CHUNKS = B_SH // CHUNK_B  # 8
SUB = CHUNK_B // 128  # 4
NFC = F_PAD // 128  # 7

F32 = mybir.dt.float32
BF16 = mybir.dt.bfloat16
AFT = mybir.ActivationFunctionType
ALU = mybir.AluOpType


def _trunc_bf16_view(ap_f32):
    """bf16 high-half (truncation) view of an f32 AP, same shape."""
    v = ap_f32.bitcast(BF16)
    if len(v.shape) == 2:
        return v.rearrange("p (n two) -> p n two", two=2)[:, :, 1]
    return v.rearrange("p q (n two) -> p q n two", two=2)[:, :, :, 1]


def build():
    nc = bacc.Bacc(
        "TRN2", target_bir_lowering=False, debug=False, num_devices=N_CORES
    )
    x_d = nc.dram_tensor("x", [B_SH, F_PAD], BF16, kind="ExternalInput").ap()
    w1t_d = nc.dram_tensor("w1t", [F_PAD, H1], BF16, kind="ExternalInput").ap()
    w2t_d = nc.dram_tensor("w2t", [H1, H2], BF16, kind="ExternalInput").ap()
    w3t_d = nc.dram_tensor("w3t", [H2, NCLS], BF16, kind="ExternalInput").ap()
    b1_d = nc.dram_tensor("b1", [H1], F32, kind="ExternalInput").ap()
    b2_d = nc.dram_tensor("b2", [H2], F32, kind="ExternalInput").ap()
    b3_d = nc.dram_tensor("b3", [NCLS], F32, kind="ExternalInput").ap()
    out_d = nc.dram_tensor("out", [B_SH, NCLS], F32, kind="ExternalOutput").ap()

    with tile.TileContext(nc) as tc:
        with (
            tc.tile_pool(name="consts", bufs=1) as consts,
            tc.tile_pool(name="xt", bufs=3) as xt_pool,
            tc.tile_pool(name="h", bufs=2) as h_pool,
            tc.tile_pool(name="lg", bufs=2) as lg_pool,
            tc.tile_pool(name="sm", bufs=8) as sm_pool,
            tc.tile_pool(name="ob", bufs=2) as ob_pool,
            tc.tile_pool(name="mm_psum", bufs=3, space="PSUM") as mm_psum,
            tc.tile_pool(name="l3_psum", bufs=2, space="PSUM") as l3_psum,
            tc.tile_pool(name="lt_psum", bufs=3, space="PSUM") as lt_psum,
        ):
            # ---- one-time: identity, weights, biases
            ident_f = consts.tile([128, 128], F32)
            make_identity(nc, ident_f[:])

            w1t = consts.tile([128, NFC, H1], BF16)
            nc.sync.dma_start(
                w1t[:], w1t_d[:].rearrange("(c k) m -> k c m", k=128)
            )
            w2t = consts.tile([128, 2, 128], BF16)
            nc.sync.dma_start(
                w2t[:], w2t_d[:].rearrange("k (c m) -> k c m", m=128)
            )
            w3t = consts.tile([128, 2, NCLS], BF16)
            nc.sync.dma_start(
                w3t[:], w3t_d[:].rearrange("(c k) m -> k c m", k=128)
            )
            b1c = consts.tile([H1, 1], F32)
            nc.sync.dma_start(b1c[:], b1_d[:])
            b2c0 = consts.tile([128, 1], F32)
            b2c1 = consts.tile([128, 1], F32)
            b2c = [b2c0, b2c1]
            for h in range(2):
                nc.sync.dma_start(b2c[h][:], b2_d[ts(h, 128)])
            b3c = consts.tile([NCLS, 1], F32)
            nc.sync.dma_start(b3c[:], b3_d[:])

            # ---- main pipeline over batch chunks of 512 rows
            for chunk in range(N_CHUNKS):
                # x chunk, transposed to feature-major by the DMA xbar
                xt = xt_pool.tile([128, NFC, CHUNK_B], BF16)
                for c in range(NFC):
                    nc.sync.dma_start_transpose(
                        xt[:, c, :], x_d[ts(chunk, CHUNK_B), ts(c, 128)]
                    )

                # L1: h1 = relu(x @ w1.T + b1)   [128 fo, 512 b]
                l1p = mm_psum.tile([128, CHUNK_B], F32, tag="mm")
                for c in range(NFC):
                    nc.tensor.matmul(
                        l1p[:],
                        w1t[:, c, :],
                        xt[:, c, :],
                        start=(c == 0),
                        stop=(c == NFC - 1),
                    )
                h1f = h_pool.tile([128, CHUNK_B], F32, tag="h1f")
                nc.scalar.activation(h1f[:], l1p[:], AFT.Relu, bias=b1c[:])
                h1b = h_pool.tile([128, CHUNK_B], BF16, tag="h1b")
                nc.gpsimd.tensor_copy(h1b[:], _trunc_bf16_view(h1f[:]))

                # L2: h2 = relu(h1 @ w2.T + b2)  [256 fo, 512 b] in halves
                h2f = h_pool.tile([128, 2, CHUNK_B], F32, tag="h2f")
                h2b = h_pool.tile([128, 2, CHUNK_B], BF16, tag="h2b")
                for h in range(2):
                    l2p = mm_psum.tile([128, CHUNK_B], F32, tag="mm")
                    nc.tensor.matmul(l2p[:], w2t[:, h, :], h1b[:])
                    nc.scalar.activation(
                        h2f[:, h, :], l2p[:], AFT.Relu, bias=b2c[h][:]
                    )
                    nc.gpsimd.tensor_copy(
                        h2b[:, h, :], _trunc_bf16_view(h2f[:, h, :])
                    )

                # L3: logits = h2 @ w3.T + b3    [10, 512]
                l3p = l3_psum.tile([NCLS, CHUNK_B], F32)
                for c in range(2):
                    nc.tensor.matmul(
                        l3p[:], w3t[:, c, :], h2b[:, c, :],
                        start=(c == 0), stop=(c == 1),
                    )
                lg = lg_pool.tile([NCLS, CHUNK_B], F32)
                nc.scalar.activation(lg[:], l3p[:], AFT.Identity, bias=b3c[:])

                # log_softmax over the 10 classes
                tcat = sm_pool.tile([128, SUB, NCLS], F32, tag="tcat")
                for s in range(SUB):
                    ltp = lt_psum.tile([128, NCLS], F32)
                    nc.tensor.transpose(
                        ltp[:], lg[:, ts(s, 128)], ident_f[0:NCLS, 0:NCLS]
                    )
                    nmax = sm_pool.tile([128, 1], F32, tag="nmax")
                    nc.vector.tensor_reduce(
                        nmax[:], ltp[:], axis=mybir.AxisListType.X,
                        op=ALU.max, negate=True,
                    )
                    nc.vector.tensor_scalar(
                        tcat[:, s, :], ltp[:], nmax[:], None, ALU.add
                    )
                ecat = sm_pool.tile([128, SUB, NCLS], F32, tag="ecat")
                nc.scalar.activation(ecat[:], tcat[:], AFT.Exp)
                ss4 = sm_pool.tile([128, SUB], F32, tag="ss4")
                nc.vector.tensor_reduce(
                    ss4[:], ecat[:], axis=mybir.AxisListType.X, op=ALU.add
                )
                ln4 = sm_pool.tile([128, SUB], F32, tag="ln4")
                nc.scalar.activation(ln4[:], ss4[:], AFT.Ln)
                ob = ob_pool.tile([128, SUB, NCLS], F32)
                for s in range(SUB):
                    nc.vector.tensor_scalar(
                        ob[:, s, :], tcat[:, s, :], ln4[:, s : s + 1], None,
                        ALU.subtract,
                    )

                odst = out_d[ts(chunk, CHUNK_B), :].rearrange(
                    "(s p) f -> p s f", p=128
                )
                nc.sync.dma_start(odst, ob[:])

    nc.compile()
    return nc


_NC_CACHE = {}


def _get_nc():
    if "nc" not in _NC_CACHE:
        _NC_CACHE["nc"] = build()
    return _NC_CACHE["nc"]


def _prep_host(x, w1, b1, w2, b2, w3, b3):
    xf = np.asarray(x, dtype=np.float32).reshape(B_FULL, F_IN)
    xb = np.zeros((B_FULL, F_PAD), dtype=ml_dtypes.bfloat16)
    xb[:, :F_IN] = xf.astype(ml_dtypes.bfloat16)
    w1tb = np.zeros((F_PAD, H1), dtype=ml_dtypes.bfloat16)
    w1tb[:F_IN, :] = np.asarray(w1, np.float32).T.astype(ml_dtypes.bfloat16)
    w2tb = np.ascontiguousarray(
        np.asarray(w2, np.float32).T.astype(ml_dtypes.bfloat16)
    )
    w3tb = np.ascontiguousarray(
        np.asarray(w3, np.float32).T.astype(ml_dtypes.bfloat16)
    )
    reps = {
        "w1t": w1tb,
        "w2t": w2tb,
        "w3t": w3tb,
        "b1": np.ascontiguousarray(np.asarray(b1, np.float32)),
        "b2": np.ascontiguousarray(np.asarray(b2, np.float32)),
        "b3": np.ascontiguousarray(np.asarray(b3, np.float32)),
    }
    return xb, reps


def kernel(x, w1, b1, w2, b2, w3, b3, _trace=False, **run_kwargs):
    nc = _get_nc()
    xb, reps = _prep_host(x, w1, b1, w2, b2, w3, b3)
    in_maps = [
        {"x": np.ascontiguousarray(xb[i * B_SH : (i + 1) * B_SH]), **reps}
        for i in range(N_CORES)
    ]
    res = run_bass_kernel_spmd(
        nc, in_maps, core_ids=list(range(N_CORES)), trace=_trace, **run_kwargs
    )
    out = np.concatenate(
        [res.results[i]["out"] for i in range(N_CORES)], axis=0
    )
    if _trace:
        return out, res
    return out


if __name__ == "__main__":
    rng = np.random.default_rng(0)
    ins = {
        "x": rng.standard_normal((B_FULL, 1, 28, 28), dtype=np.float32),
        "w1": rng.standard_normal((H1, F_IN), dtype=np.float32),
        "b1": rng.standard_normal((H1,), dtype=np.float32),
        "w2": rng.standard_normal((H2, H1), dtype=np.float32),
        "b2": rng.standard_normal((H2,), dtype=np.float32),
        "w3": rng.standard_normal((NCLS, H2), dtype=np.float32),
        "b3": rng.standard_normal((NCLS,), dtype=np.float32),
    }
    out = kernel(**ins)
    print("out:", out.shape, out.dtype)
